# revision 73
# baseline (speedup 1.0000x reference)
"""Trainium2 Bass kernel for nn_FFN_pairwise_z (pairwise-concat FFN scoring).

Math (see reference):
    a = op @ W1[:z]           [N_op, h]
    b = co @ W1[z:]           [N_co, h]
    score_ij = relu( relu(a_i + b_j + b1) . W2 + b2 )
    OP_w[i] = sum_j score, CO_w[j] = sum_i score, T = sum_ij score
    out = (OP_w @ op / T,  CO_w @ co / T)       two [1, z] vectors

Sharding: N_op rows split across 8 cores (128 rows each).  Each core
computes its score block [128, 1024] without materializing it in DRAM and
emits only partial sums:
    u_op_part   = OP_w_local @ op_local        [z]
    T_part      = sum(OP_w_local)              [1]
    u_co_part   = CO_w_part @ co               [z]
packed as one [1, 2z+1] output.  The host adds the 8 partials and divides
by T (the "all-reduce + normalize" step of the hinted strategy, done on
host since it is 257 floats).

Device pipeline per core (layout: h on partitions):
    bT   = (co @ W1b)^T     [h=128, N_co]   fp16, via 2 fp32 matmuls
    abias= (op_l @ W1a)^T + b1  [h, 128]    fp32
    per i in 0..127:
        hid_i = max(bT + abias[:, i], 0)    one DVE tensor_scalar (fp16, 4x)
        s[i, :] = W2^T @ hid_i              two fp16 matmuls -> PSUM row i
    score = relu(s + b2) (ACT, accum_out gives OP_w_local for free)
    u_op|T  : one matmul  lhsT=OP_w_local, rhs=[op_l | ones]
    CO_w^T  : 8 matmuls   lhsT=score chunk, rhs=ones
    u_co    : 8 accumulating matmuls lhsT=CO_w^T col, rhs=co chunk
"""

import os
import sys

for _p in ("/opt/trn_rl_repo", "/root/.axon_site/_ro/trn_rl_repo"):
    if os.path.isdir(_p) and _p not in sys.path:
        sys.path.insert(0, _p)

import numpy as np

import concourse.bacc as bacc
import concourse.tile as tile
from concourse import mybir
from concourse.bass_utils import run_bass_kernel_spmd

N_OP, N_CO, Z, H = 1024, 1024, 128, 128
N_CORES = 8
ROWS = N_OP // N_CORES  # 128 op-rows per core
F32 = mybir.dt.float32
F16 = mybir.dt.float16
F8 = mybir.dt.float8e4
OUT_W = 2 * Z + 1  # u_op (z) | T (1) | u_co (z)

_CACHE = {}
LAST_EXEC_NS = None


def _build():
    nc = bacc.Bacc("TRN2", target_bir_lowering=False, debug=False)

    op_ext = nc.dram_tensor("op_ext", [ROWS, Z + 1], F16, kind="ExternalInput")
    coT = nc.dram_tensor("coT", [Z, N_CO], F16, kind="ExternalInput")
    co_pk = nc.dram_tensor("co_pk", [128, N_CO], F16, kind="ExternalInput")
    # w1b | w1a | op_lT | W2 col | b1 col | b2 col packed as one fp16 tensor
    WPW = 2 * H + ROWS + 3
    wpack = nc.dram_tensor("wpack", [Z, WPW], F16, kind="ExternalInput")
    out = nc.dram_tensor("out", [1, OUT_W], F32, kind="ExternalOutput")

    with tile.TileContext(nc) as tc:
        with (
            tc.tile_pool(name="singles", bufs=1) as singles,
            tc.tile_pool(name="hidp", bufs=4) as hidp,
            tc.tile_pool(name="hidp8", bufs=4) as hidp8,
            tc.tile_pool(name="ps_main", bufs=1, space="PSUM") as psm,
            tc.tile_pool(name="ps_bt", bufs=2, space="PSUM") as psbt,
            tc.tile_pool(name="ps_tmp", bufs=1, space="PSUM") as pst,
        ):
            # PE warmup: dummy matmuls during the DMA window get the HAM
            # clock gate to 8/8 before the preamble/main matmuls run.
            sb_warm = singles.tile([128, 128], F16)
            nc.vector.memset(sb_warm[:, :], 0.0)
            ps_warm = pst.tile([128, 1], F32, tag="opw")
            for _ in range(20):
                nc.tensor.matmul(
                    ps_warm[:, :], lhsT=sb_warm[:, :], rhs=sb_warm[:, 0:1],
                    start=True, stop=True,
                )

            # 5 input DMAs spread over the two hwdge queues, critical first.
            sb_wpack = singles.tile([128, WPW], F16)
            nc.sync.dma_start(out=sb_wpack[:, :], in_=wpack[:, :])
            sb_coT = singles.tile([128, N_CO], F16)
            nc.gpsimd.dma_start(out=sb_coT[:, 0:512], in_=coT[:, 0:512])
            nc.sync.dma_start(out=sb_coT[:, 512:1024], in_=coT[:, 512:1024])
            sb_copk = singles.tile([128, N_CO], F16)
            nc.gpsimd.dma_start(out=sb_copk[:, :], in_=co_pk[:, :])
            sb_opext = singles.tile([128, Z + 1], F16)
            nc.gpsimd.dma_start(out=sb_opext[:, :], in_=op_ext[:, :])
            sb_w1b = sb_wpack[:, 0:H]
            sb_w1a = sb_wpack[:, H : 2 * H]
            sb_oplT = sb_wpack[:, 2 * H : 2 * H + ROWS]
            sb_w2 = sb_wpack[:, 2 * H + ROWS : 2 * H + ROWS + 1]
            sb_b1c16 = sb_wpack[:, 2 * H + ROWS + 1 : 2 * H + ROWS + 2]
            sb_b2c16 = sb_wpack[:, 2 * H + ROWS + 2 : 2 * H + ROWS + 3]

            # fp32 copies of the b1/b2 columns (TS scalars must be fp32)
            sb_b1 = singles.tile([128, 1], F32)
            nc.vector.tensor_copy(sb_b1[:, :], sb_b1c16)
            sb_b2 = singles.tile([128, 1], F32)
            nc.vector.tensor_copy(sb_b2[:, :], sb_b2c16)

            # abias[h, i] = sum_z W1a[z,h] opT[z,i]; b1 folded in during the
            # PSUM drain (tensor_scalar_add with the b1 column as scalar).
            ps_a = pst.tile([128, ROWS], F32, tag="small")
            nc.tensor.matmul(ps_a[:, :], lhsT=sb_w1a[:, :], rhs=sb_oplT[:, :], start=True, stop=True)
            sb_abias = singles.tile([128, ROWS], F32)
            nc.vector.tensor_scalar_add(out=sb_abias[:, :], in0=ps_a[:, :], scalar1=sb_b1[:, 0:1])

            # bT[h, j] = sum_z W1b[z, h] * coT[z, j], stored fp16
            sb_bT = singles.tile([128, N_CO], F16)
            for half in range(2):
                ps_b = psbt.tile([128, 512], F32, tag="bt")
                nc.tensor.matmul(
                    ps_b[:, :],
                    lhsT=sb_w1b[:, :],
                    rhs=sb_coT[:, half * 512 : (half + 1) * 512],
                    start=True,
                    stop=True,
                )
                if half == 0:
                    # first half drained by DVE so its own TS rows unblock
                    # as soon as possible; second half on ACT in parallel
                    nc.vector.tensor_copy(sb_bT[:, 0:512], ps_b[:, :])
                else:
                    nc.scalar.copy(sb_bT[:, 512:1024], ps_b)

            # main pairwise loop.  hid chunks go through the PE as the
            # STATIONARY operand (fp16 weight loads stream 2 elem/cycle),
            # W2 as the moving operand (N=1): one [128,1] psum column per
            # (i, j-chunk).  Scores land in one PSUM tile per 32-row group
            # (layout [j, c*32 + i%32]) so drains + CO_w partials for groups
            # 0-2 run hidden under the main loop.
            # uneven groups: the small last group is the only drain+reduce
            # on the critical path after the loop, so keep it short
            N_GRP = 4
            GRP_SIZES = (32, 32, 48, 16)
            GRP_STARTS = (0, 32, 64, 112)

            def grp_of(i):
                for g in range(N_GRP - 1, -1, -1):
                    if i >= GRP_STARTS[g]:
                        return g, i - GRP_STARTS[g]
                raise AssertionError

            # padded to a full 2KiB PSUM bank each so no two tiles share a
            # bank (PE-write + DVE-read on one bank is fatal)
            gt = [
                psm.tile(
                    [128, 8, GRP_SIZES[g]], F32, tag=f"g{g}", name=f"gt{g}",
                    padded_shape=[128, 8, 64],
                )
                for g in range(N_GRP)
            ]
            CORD = (0, 4, 1, 5, 2, 6, 3, 7)  # alternate PSUM banks
            # 8 hid rows per super-tile (one sync group) to cut per-row
            # semaphore traffic 8x.  Rows 0-5 on DVE (~330-400ns each), rows
            # 6-7 on ACT (~1040ns) - measured back-to-back paces.
            SUP = 8
            for s in range(ROWS // SUP):
                hid = hidp.tile([128, SUP * N_CO], F16, tag="hid")
                # ACT-produced rows go to a separate fp8 tile: ACT runs at
                # 1x regardless of dtype, and the PE's fast-weight-load
                # streams fp8 at twice the fp16 rate, so those rows' weight
                # loads halve and the PE tracks row production instead of
                # accumulating a multi-us backlog it must burn in the tail.
                n_act = 3 if s in (4, 9, 14) else 2
                hid8 = hidp8.tile([128, 3 * N_CO], F8, tag="hid8")
                for r in range(SUP):
                    i = s * SUP + r
                    dst = hid[:, r * N_CO : (r + 1) * N_CO]
                    # 93 DVE / 35 ACT rows; extra ACT rows kept away from the
                    # last supers so the loop end stays DVE(fast)-paced
                    if r >= SUP - n_act:
                        k = r - (SUP - n_act)
                        nc.scalar.activation(
                            out=hid8[:, k * N_CO : (k + 1) * N_CO],
                            in_=sb_bT[:, :],
                            func=mybir.ActivationFunctionType.Relu,
                            bias=sb_abias[:, i : i + 1],
                        )
                    elif s == 0 and r < 2:
                        # split over the bT halves so the loop starts as soon
                        # as the first half of bT is drained from PSUM
                        for hf in range(2):
                            nc.vector.tensor_scalar(
                                out=dst[:, hf * 512 : (hf + 1) * 512],
                                in0=sb_bT[:, hf * 512 : (hf + 1) * 512],
                                scalar1=sb_abias[:, i : i + 1],
                                scalar2=0.0,
                                op0=mybir.AluOpType.add,
                                op1=mybir.AluOpType.max,
                            )
                    else:
                        nc.vector.tensor_scalar(
                            out=dst,
                            in0=sb_bT[:, :],
                            scalar1=sb_abias[:, i : i + 1],
                            scalar2=0.0,
                            op0=mybir.AluOpType.add,
                            op1=mybir.AluOpType.max,
                        )
                for r in range(SUP):
                    i = s * SUP + r
                    g, iw = grp_of(i)
                    if r >= SUP - n_act:
                        k = r - (SUP - n_act)
                        src = hid8[:, k * N_CO : (k + 1) * N_CO]
                    else:
                        src = hid[:, r * N_CO : (r + 1) * N_CO]
                    for c in CORD:
                        nc.tensor.matmul(
                            gt[g][:, c, iw : iw + 1],
                            lhsT=src[:, c * 128 : (c + 1) * 128],
                            rhs=sb_w2[:, :],
                            start=True,
                            stop=True,
                        )

            # Per-group drain: ACT applies relu(.+b2) writing scoreT in
            # (c, i)-major layout via 3D APs; DVE reduces per-group CO_w
            # partials (straight to fp16 - the u_co lhsT dtype).
            sb_scoreT = singles.tile([128, 8, 128], F16)  # [j, c, i]
            sb_cwp = singles.tile([128, 8, N_GRP], F16)   # [j, c, g]
            sb_cw16 = singles.tile([128, 8], F16)
            sb_one16 = singles.tile([128, 1], F16)
            nc.vector.memset(sb_one16[:, :], 1.0)
            ps_opw = pst.tile([128, 1], F32, tag="opw")
            # u_op | T | u_co packed in one PSUM bank -> single copy out.
            # Reuses a bT-pool slot (free after the preamble) to stay in 8 banks.
            ps_res = psbt.tile([1, OUT_W], F32, tag="bt")
            for g in range(N_GRP):
                g0, g1 = GRP_STARTS[g], GRP_STARTS[g] + GRP_SIZES[g]
                nc.scalar.activation(
                    out=sb_scoreT[:, :, g0:g1],
                    in_=gt[g][:, :, :],
                    func=mybir.ActivationFunctionType.Relu,
                    bias=sb_b2[:, :],
                )
                with nc.allow_low_precision(reason="CO_w partials: <=48 adds of O(1) fp16 scores, tol 2e-2"):
                    nc.vector.tensor_reduce(
                        op=mybir.AluOpType.add,
                        out=sb_cwp[:, :, g : g + 1],
                        in_=sb_scoreT[:, :, g0:g1],
                        axis=mybir.AxisListType.X,
                    )
            # sum the 4 group partials per chunk (innermost dim = g)
            with nc.allow_low_precision(reason="CO_w group-partial sum: 4 adds, tol 2e-2"):
                nc.vector.tensor_reduce(
                    op=mybir.AluOpType.add,
                    out=sb_cw16[:, :],
                    in_=sb_cwp[:, :, :],
                    axis=mybir.AxisListType.X,
                )
            for c in range(8):
                nc.tensor.matmul(
                    ps_opw[:, :],
                    lhsT=sb_scoreT[:, c, :],
                    rhs=sb_one16[:, :],
                    start=(c == 0),
                    stop=(c == 7),
                )
                nc.tensor.matmul(
                    ps_res[0:1, Z + 1 : OUT_W],
                    lhsT=sb_cw16[:, c : c + 1],
                    rhs=sb_copk[:, c * 128 : (c + 1) * 128],
                    start=(c == 0),
                    stop=(c == 7),
                )
            sb_opw16 = singles.tile([128, 1], F16)
            nc.vector.tensor_copy(sb_opw16[:, :], ps_opw[:, :])

            # u_op | T  (T via the ones column appended to op_ext)
            nc.tensor.matmul(ps_res[0:1, 0 : Z + 1], lhsT=sb_opw16[:, :], rhs=sb_opext[:, :], start=True, stop=True)

            # split output: u_op|T copies+ships (sync queue) while the u_co
            # matmuls are still accumulating; u_co follows on the gpsimd
            # queue (idle by now) so the two pushes overlap
            sb_out = singles.tile([1, OUT_W], F32)
            nc.vector.tensor_copy(sb_out[0:1, 0 : Z + 1], ps_res[0:1, 0 : Z + 1])
            nc.sync.dma_start(out=out[:, 0 : Z + 1], in_=sb_out[0:1, 0 : Z + 1])
            nc.vector.tensor_copy(sb_out[0:1, Z + 1 : OUT_W], ps_res[0:1, Z + 1 : OUT_W])
            nc.gpsimd.dma_start(out=out[:, Z + 1 : OUT_W], in_=sb_out[0:1, Z + 1 : OUT_W])

    nc.compile()
    return nc


def _make_in_maps(OP_zs, CO_zs, W1, b1, W2, b2):
    op = np.asarray(OP_zs, dtype=np.float32)[0]  # [N_op, z]
    co = np.asarray(CO_zs, dtype=np.float32)[0]  # [N_co, z]
    W1 = np.asarray(W1, dtype=np.float32)
    b1 = np.asarray(b1, dtype=np.float32)
    W2 = np.asarray(W2, dtype=np.float32)
    b2 = np.asarray(b2, dtype=np.float32)

    coT = np.ascontiguousarray(co.T.astype(np.float16))  # [z, N_co]
    co_pk = np.ascontiguousarray(
        co.reshape(8, 128, Z).transpose(1, 0, 2).reshape(128, 8 * Z)
    ).astype(np.float16)  # [p, t*z] : row p holds co[t*128+p, :] for t=0..7
    shared = {
        "coT": coT,
        "co_pk": co_pk,
    }
    w1b16 = W1[Z:].astype(np.float16)
    w1a16 = W1[:Z].astype(np.float16)
    w2col = W2.astype(np.float16)[:, None]
    b1col = b1.astype(np.float16)[:, None]
    b2col = np.full((Z, 1), b2[0], dtype=np.float16)
    in_maps = []
    for c in range(N_CORES):
        opc = op[c * ROWS : (c + 1) * ROWS]
        in_maps.append(
            {
                **shared,
                "op_ext": np.ascontiguousarray(
                    np.concatenate(
                        [opc, np.ones((ROWS, 1), dtype=np.float32)], axis=1
                    ).astype(np.float16)
                ),
                "wpack": np.ascontiguousarray(
                    np.concatenate(
                        [w1b16, w1a16, opc.T.astype(np.float16), w2col, b1col, b2col],
                        axis=1,
                    )
                ),
            }
        )
    return in_maps


def _ensure_ntff_hook():
    """This image's antenv lacks axon_hooks; synthesize it so trace=True can
    drive NTFF profiling via the axon .so (profiling-only, dev-loop)."""
    import types

    try:
        from antenv.axon_hooks import get_axon_ntff_profile_hook  # noqa: F401

        return True
    except ImportError:
        pass
    try:
        sys.path.insert(0, "/root/.axon_site")
        from trn_agent_boot.trn_boot import _ntff_profile_via_ctypes

        hook = _ntff_profile_via_ctypes("/opt/axon/libaxon_pjrt.so")
        if hook is None:
            return False
        import antenv

        mod = types.ModuleType("antenv.axon_hooks")
        _state = {"hook": hook}
        mod.set_axon_ntff_profile_hook = lambda h: _state.__setitem__("hook", h)
        mod.get_axon_ntff_profile_hook = lambda: _state["hook"]
        sys.modules["antenv.axon_hooks"] = mod
        antenv.axon_hooks = mod
        return True
    except Exception as e:  # pragma: no cover - profiling is best-effort
        print(f"ntff hook setup failed: {e}")
        return False


def kernel(OP_zs, CO_zs, W1, b1, W2, b2):
    global LAST_EXEC_NS
    if "nc" not in _CACHE:
        _CACHE["nc"] = _build()
    nc = _CACHE["nc"]
    in_maps = _make_in_maps(OP_zs, CO_zs, W1, b1, W2, b2)

    trace = bool(os.environ.get("KERNEL_PROFILE"))
    if trace:
        trace = _ensure_ntff_hook()
    res = run_bass_kernel_spmd(nc, in_maps, list(range(N_CORES)), trace=trace)
    if getattr(res, "exec_time_ns", None) is not None:
        LAST_EXEC_NS = res.exec_time_ns

    u = np.zeros(OUT_W, dtype=np.float64)
    for r in res.results:
        u += r["out"][0].astype(np.float64)
    u_op, T, u_co = u[0:Z], u[Z], u[Z + 1 :]

    if T == 0.0:
        # all-scores-zero fallback: reproduce the reference's jax.random draw
        import jax

        with jax.default_device(jax.devices("cpu")[0]):
            k = jax.random.key(1)
            OP_w = np.asarray(jax.random.uniform(k, (N_OP,)), dtype=np.float64)
            CO_w = np.asarray(
                jax.random.uniform(jax.random.fold_in(k, 1), (N_CO,)),
                dtype=np.float64,
            )
        op = np.asarray(OP_zs, dtype=np.float64)[0]
        co = np.asarray(CO_zs, dtype=np.float64)[0]
        u_op, u_co = OP_w @ op, CO_w @ co
        return (
            (u_op / OP_w.sum())[None].astype(np.float32),
            (u_co / CO_w.sum())[None].astype(np.float32),
        )

    return (
        (u_op / T)[None].astype(np.float32),
        (u_co / T)[None].astype(np.float32),
    )


# revision 74
# speedup vs baseline: 1.0422x; 1.0422x over previous
"""Trainium2 Bass kernel for nn_FFN_pairwise_z (pairwise-concat FFN scoring).

Math (see reference):
    a = op @ W1[:z]           [N_op, h]
    b = co @ W1[z:]           [N_co, h]
    score_ij = relu( relu(a_i + b_j + b1) . W2 + b2 )
    OP_w[i] = sum_j score, CO_w[j] = sum_i score, T = sum_ij score
    out = (OP_w @ op / T,  CO_w @ co / T)       two [1, z] vectors

Sharding: N_op rows split across 8 cores (128 rows each).  Each core
computes its score block [128, 1024] without materializing it in DRAM and
emits only partial sums:
    u_op_part   = OP_w_local @ op_local        [z]
    T_part      = sum(OP_w_local)              [1]
    u_co_part   = CO_w_part @ co               [z]
packed as one [1, 2z+1] output.  The host adds the 8 partials and divides
by T (the "all-reduce + normalize" step of the hinted strategy, done on
host since it is 257 floats).

Device pipeline per core (layout: h on partitions):
    bT   = (co @ W1b)^T     [h=128, N_co]   fp16, via 2 fp32 matmuls
    abias= (op_l @ W1a)^T + b1  [h, 128]    fp32
    per i in 0..127:
        hid_i = max(bT + abias[:, i], 0)    one DVE tensor_scalar (fp16, 4x)
        s[i, :] = W2^T @ hid_i              two fp16 matmuls -> PSUM row i
    score = relu(s + b2) (ACT, accum_out gives OP_w_local for free)
    u_op|T  : one matmul  lhsT=OP_w_local, rhs=[op_l | ones]
    CO_w^T  : 8 matmuls   lhsT=score chunk, rhs=ones
    u_co    : 8 accumulating matmuls lhsT=CO_w^T col, rhs=co chunk
"""

import os
import sys

for _p in ("/opt/trn_rl_repo", "/root/.axon_site/_ro/trn_rl_repo"):
    if os.path.isdir(_p) and _p not in sys.path:
        sys.path.insert(0, _p)

import numpy as np

import concourse.bacc as bacc
import concourse.tile as tile
from concourse import mybir
from concourse.bass_utils import run_bass_kernel_spmd

N_OP, N_CO, Z, H = 1024, 1024, 128, 128
N_CORES = 8
ROWS = N_OP // N_CORES  # 128 op-rows per core
F32 = mybir.dt.float32
F16 = mybir.dt.float16
F8 = mybir.dt.float8e4
OUT_W = 2 * Z + 1  # u_op (z) | T (1) | u_co (z)

_CACHE = {}
LAST_EXEC_NS = None


def _build():
    nc = bacc.Bacc("TRN2", target_bir_lowering=False, debug=False)

    op_ext = nc.dram_tensor("op_ext", [ROWS, Z + 1], F16, kind="ExternalInput")
    coT = nc.dram_tensor("coT", [Z, N_CO], F16, kind="ExternalInput")
    co_pk = nc.dram_tensor("co_pk", [128, N_CO], F16, kind="ExternalInput")
    # w1b | w1a | op_lT | W2 col | b1 col | b2 col packed as one fp16 tensor
    WPW = 2 * H + ROWS + 3
    wpack = nc.dram_tensor("wpack", [Z, WPW], F16, kind="ExternalInput")
    out = nc.dram_tensor("out", [1, OUT_W], F32, kind="ExternalOutput")

    with tile.TileContext(nc) as tc:
        with (
            tc.tile_pool(name="singles", bufs=1) as singles,
            tc.tile_pool(name="hidp", bufs=4) as hidp,
            tc.tile_pool(name="hidp8", bufs=4) as hidp8,
            tc.tile_pool(name="ps_main", bufs=1, space="PSUM") as psm,
            tc.tile_pool(name="ps_bt", bufs=2, space="PSUM") as psbt,
            tc.tile_pool(name="ps_tmp", bufs=1, space="PSUM") as pst,
        ):
            # PE warmup: dummy matmuls during the DMA window get the HAM
            # clock gate to 8/8 before the preamble/main matmuls run.
            sb_warm = singles.tile([128, 128], F16)
            nc.vector.memset(sb_warm[:, :], 0.0)
            ps_warm = pst.tile([128, 1], F32, tag="opw")
            for _ in range(20):
                nc.tensor.matmul(
                    ps_warm[:, :], lhsT=sb_warm[:, :], rhs=sb_warm[:, 0:1],
                    start=True, stop=True,
                )

            # 5 input DMAs spread over the two hwdge queues, critical first.
            sb_wpack = singles.tile([128, WPW], F16)
            nc.sync.dma_start(out=sb_wpack[:, :], in_=wpack[:, :])
            sb_coT = singles.tile([128, N_CO], F16)
            nc.gpsimd.dma_start(out=sb_coT[:, 0:512], in_=coT[:, 0:512])
            nc.sync.dma_start(out=sb_coT[:, 512:1024], in_=coT[:, 512:1024])
            sb_copk = singles.tile([128, N_CO], F16)
            nc.gpsimd.dma_start(out=sb_copk[:, :], in_=co_pk[:, :])
            sb_opext = singles.tile([128, Z + 1], F16)
            nc.gpsimd.dma_start(out=sb_opext[:, :], in_=op_ext[:, :])
            sb_w1b = sb_wpack[:, 0:H]
            sb_w1a = sb_wpack[:, H : 2 * H]
            sb_oplT = sb_wpack[:, 2 * H : 2 * H + ROWS]
            sb_w2 = sb_wpack[:, 2 * H + ROWS : 2 * H + ROWS + 1]
            sb_b1c16 = sb_wpack[:, 2 * H + ROWS + 1 : 2 * H + ROWS + 2]
            sb_b2c16 = sb_wpack[:, 2 * H + ROWS + 2 : 2 * H + ROWS + 3]

            # fp32 copies of the b1/b2 columns (TS scalars must be fp32)
            sb_b1 = singles.tile([128, 1], F32)
            nc.vector.tensor_copy(sb_b1[:, :], sb_b1c16)
            sb_b2 = singles.tile([128, 1], F32)
            nc.vector.tensor_copy(sb_b2[:, :], sb_b2c16)

            # abias[h, i] = sum_z W1a[z,h] opT[z,i]; b1 folded in during the
            # PSUM drain (tensor_scalar_add with the b1 column as scalar).
            ps_a = pst.tile([128, ROWS], F32, tag="small")
            nc.tensor.matmul(ps_a[:, :], lhsT=sb_w1a[:, :], rhs=sb_oplT[:, :], start=True, stop=True)
            sb_abias = singles.tile([128, ROWS], F32)
            nc.vector.tensor_scalar_add(out=sb_abias[:, :], in0=ps_a[:, :], scalar1=sb_b1[:, 0:1])

            # bT[h, j] = sum_z W1b[z, h] * coT[z, j], stored fp16
            sb_bT = singles.tile([128, N_CO], F16)
            for half in range(2):
                ps_b = psbt.tile([128, 512], F32, tag="bt")
                nc.tensor.matmul(
                    ps_b[:, :],
                    lhsT=sb_w1b[:, :],
                    rhs=sb_coT[:, half * 512 : (half + 1) * 512],
                    start=True,
                    stop=True,
                )
                if half == 0:
                    # first half drained by DVE so its own TS rows unblock
                    # as soon as possible; second half on ACT in parallel
                    nc.vector.tensor_copy(sb_bT[:, 0:512], ps_b[:, :])
                else:
                    nc.scalar.copy(sb_bT[:, 512:1024], ps_b)

            # main pairwise loop.  hid chunks go through the PE as the
            # STATIONARY operand (fp16 weight loads stream 2 elem/cycle),
            # W2 as the moving operand (N=1): one [128,1] psum column per
            # (i, j-chunk).  Scores land in one PSUM tile per 32-row group
            # (layout [j, c*32 + i%32]) so drains + CO_w partials for groups
            # 0-2 run hidden under the main loop.
            N_GRP = 4
            GRP = ROWS // N_GRP  # 32 rows per group
            # padded to a full 2KiB PSUM bank each so no two tiles share a
            # bank (PE-write + DVE-read on one bank is fatal)
            gt = [
                psm.tile(
                    [128, 8, GRP], F32, tag=f"g{g}", name=f"gt{g}",
                    padded_shape=[128, 8, 64],
                )
                for g in range(N_GRP)
            ]
            CORD = (0, 4, 1, 5, 2, 6, 3, 7)  # alternate PSUM banks
            # 8 hid rows per super-tile (one sync group) to cut per-row
            # semaphore traffic 8x.  Rows 0-5 on DVE (~330-400ns each), rows
            # 6-7 on ACT (~1040ns) - measured back-to-back paces.
            SUP = 8
            for s in range(ROWS // SUP):
                hid = hidp.tile([128, SUP * N_CO], F16, tag="hid")
                # ACT-produced rows go to a separate fp8 tile: ACT runs at
                # 1x regardless of dtype, and the PE's fast-weight-load
                # streams fp8 at twice the fp16 rate, so those rows' weight
                # loads halve and the PE tracks row production instead of
                # accumulating a multi-us backlog it must burn in the tail.
                n_act = 3 if s in (4, 9, 14) else 2
                hid8 = hidp8.tile([128, 3 * N_CO], F8, tag="hid8")
                for r in range(SUP):
                    i = s * SUP + r
                    dst = hid[:, r * N_CO : (r + 1) * N_CO]
                    # 93 DVE / 35 ACT rows; extra ACT rows kept away from the
                    # last supers so the loop end stays DVE(fast)-paced
                    if r >= SUP - n_act:
                        k = r - (SUP - n_act)
                        nc.scalar.activation(
                            out=hid8[:, k * N_CO : (k + 1) * N_CO],
                            in_=sb_bT[:, :],
                            func=mybir.ActivationFunctionType.Relu,
                            bias=sb_abias[:, i : i + 1],
                        )
                    elif s == 0 and r < 2:
                        # split over the bT halves so the loop starts as soon
                        # as the first half of bT is drained from PSUM
                        for hf in range(2):
                            nc.vector.tensor_scalar(
                                out=dst[:, hf * 512 : (hf + 1) * 512],
                                in0=sb_bT[:, hf * 512 : (hf + 1) * 512],
                                scalar1=sb_abias[:, i : i + 1],
                                scalar2=0.0,
                                op0=mybir.AluOpType.add,
                                op1=mybir.AluOpType.max,
                            )
                    else:
                        nc.vector.tensor_scalar(
                            out=dst,
                            in0=sb_bT[:, :],
                            scalar1=sb_abias[:, i : i + 1],
                            scalar2=0.0,
                            op0=mybir.AluOpType.add,
                            op1=mybir.AluOpType.max,
                        )
                for r in range(SUP):
                    i = s * SUP + r
                    g, iw = i // GRP, i % GRP
                    if r >= SUP - n_act:
                        k = r - (SUP - n_act)
                        src = hid8[:, k * N_CO : (k + 1) * N_CO]
                    else:
                        src = hid[:, r * N_CO : (r + 1) * N_CO]
                    for c in CORD:
                        nc.tensor.matmul(
                            gt[g][:, c, iw : iw + 1],
                            lhsT=src[:, c * 128 : (c + 1) * 128],
                            rhs=sb_w2[:, :],
                            start=True,
                            stop=True,
                        )

            # Per-group drain: ACT applies relu(.+b2) writing scoreT in
            # (c, i)-major layout via 3D APs; DVE reduces per-group CO_w
            # partials (straight to fp16 - the u_co lhsT dtype).
            sb_scoreT = singles.tile([128, 8, 128], F16)  # [j, c, i]
            sb_cwp = singles.tile([128, 8, N_GRP], F16)   # [j, c, g]
            sb_cw16 = singles.tile([128, 8], F16)
            sb_one16 = singles.tile([128, 1], F16)
            nc.vector.memset(sb_one16[:, :], 1.0)
            ps_opw = pst.tile([128, 1], F32, tag="opw")
            # u_op | T | u_co packed in one PSUM bank -> single copy out.
            # Reuses a bT-pool slot (free after the preamble) to stay in 8 banks.
            ps_res = psbt.tile([1, OUT_W], F32, tag="bt")
            for g in range(N_GRP):
                nc.scalar.activation(
                    out=sb_scoreT[:, :, g * GRP : (g + 1) * GRP],
                    in_=gt[g][:, :, :],
                    func=mybir.ActivationFunctionType.Relu,
                    bias=sb_b2[:, :],
                )
                with nc.allow_low_precision(reason="CO_w partials: 32 adds of O(1) fp16 scores, tol 2e-2"):
                    nc.vector.tensor_reduce(
                        op=mybir.AluOpType.add,
                        out=sb_cwp[:, :, g : g + 1],
                        in_=sb_scoreT[:, :, g * GRP : (g + 1) * GRP],
                        axis=mybir.AxisListType.X,
                    )
            # sum the 4 group partials per chunk (innermost dim = g)
            with nc.allow_low_precision(reason="CO_w group-partial sum: 4 adds, tol 2e-2"):
                nc.vector.tensor_reduce(
                    op=mybir.AluOpType.add,
                    out=sb_cw16[:, :],
                    in_=sb_cwp[:, :, :],
                    axis=mybir.AxisListType.X,
                )
            for c in range(8):
                nc.tensor.matmul(
                    ps_opw[:, :],
                    lhsT=sb_scoreT[:, c, :],
                    rhs=sb_one16[:, :],
                    start=(c == 0),
                    stop=(c == 7),
                )
                nc.tensor.matmul(
                    ps_res[0:1, Z + 1 : OUT_W],
                    lhsT=sb_cw16[:, c : c + 1],
                    rhs=sb_copk[:, c * 128 : (c + 1) * 128],
                    start=(c == 0),
                    stop=(c == 7),
                )
            sb_opw16 = singles.tile([128, 1], F16)
            nc.vector.tensor_copy(sb_opw16[:, :], ps_opw[:, :])

            # u_op | T  (T via the ones column appended to op_ext)
            nc.tensor.matmul(ps_res[0:1, 0 : Z + 1], lhsT=sb_opw16[:, :], rhs=sb_opext[:, :], start=True, stop=True)

            sb_out = singles.tile([1, OUT_W], F32)
            nc.vector.tensor_copy(sb_out[0:1, :], ps_res[0:1, :])
            nc.sync.dma_start(out=out[:, :], in_=sb_out[0:1, :])

    nc.compile()
    return nc


def _make_in_maps(OP_zs, CO_zs, W1, b1, W2, b2):
    op = np.asarray(OP_zs, dtype=np.float32)[0]  # [N_op, z]
    co = np.asarray(CO_zs, dtype=np.float32)[0]  # [N_co, z]
    W1 = np.asarray(W1, dtype=np.float32)
    b1 = np.asarray(b1, dtype=np.float32)
    W2 = np.asarray(W2, dtype=np.float32)
    b2 = np.asarray(b2, dtype=np.float32)

    coT = np.ascontiguousarray(co.T.astype(np.float16))  # [z, N_co]
    co_pk = np.ascontiguousarray(
        co.reshape(8, 128, Z).transpose(1, 0, 2).reshape(128, 8 * Z)
    ).astype(np.float16)  # [p, t*z] : row p holds co[t*128+p, :] for t=0..7
    shared = {
        "coT": coT,
        "co_pk": co_pk,
    }
    w1b16 = W1[Z:].astype(np.float16)
    w1a16 = W1[:Z].astype(np.float16)
    w2col = W2.astype(np.float16)[:, None]
    b1col = b1.astype(np.float16)[:, None]
    b2col = np.full((Z, 1), b2[0], dtype=np.float16)
    in_maps = []
    for c in range(N_CORES):
        opc = op[c * ROWS : (c + 1) * ROWS]
        in_maps.append(
            {
                **shared,
                "op_ext": np.ascontiguousarray(
                    np.concatenate(
                        [opc, np.ones((ROWS, 1), dtype=np.float32)], axis=1
                    ).astype(np.float16)
                ),
                "wpack": np.ascontiguousarray(
                    np.concatenate(
                        [w1b16, w1a16, opc.T.astype(np.float16), w2col, b1col, b2col],
                        axis=1,
                    )
                ),
            }
        )
    return in_maps


def _ensure_ntff_hook():
    """This image's antenv lacks axon_hooks; synthesize it so trace=True can
    drive NTFF profiling via the axon .so (profiling-only, dev-loop)."""
    import types

    try:
        from antenv.axon_hooks import get_axon_ntff_profile_hook  # noqa: F401

        return True
    except ImportError:
        pass
    try:
        sys.path.insert(0, "/root/.axon_site")
        from trn_agent_boot.trn_boot import _ntff_profile_via_ctypes

        hook = _ntff_profile_via_ctypes("/opt/axon/libaxon_pjrt.so")
        if hook is None:
            return False
        import antenv

        mod = types.ModuleType("antenv.axon_hooks")
        _state = {"hook": hook}
        mod.set_axon_ntff_profile_hook = lambda h: _state.__setitem__("hook", h)
        mod.get_axon_ntff_profile_hook = lambda: _state["hook"]
        sys.modules["antenv.axon_hooks"] = mod
        antenv.axon_hooks = mod
        return True
    except Exception as e:  # pragma: no cover - profiling is best-effort
        print(f"ntff hook setup failed: {e}")
        return False


def kernel(OP_zs, CO_zs, W1, b1, W2, b2):
    global LAST_EXEC_NS
    if "nc" not in _CACHE:
        _CACHE["nc"] = _build()
    nc = _CACHE["nc"]
    in_maps = _make_in_maps(OP_zs, CO_zs, W1, b1, W2, b2)

    trace = bool(os.environ.get("KERNEL_PROFILE"))
    if trace:
        trace = _ensure_ntff_hook()
    res = run_bass_kernel_spmd(nc, in_maps, list(range(N_CORES)), trace=trace)
    if getattr(res, "exec_time_ns", None) is not None:
        LAST_EXEC_NS = res.exec_time_ns

    u = np.zeros(OUT_W, dtype=np.float64)
    for r in res.results:
        u += r["out"][0].astype(np.float64)
    u_op, T, u_co = u[0:Z], u[Z], u[Z + 1 :]

    if T == 0.0:
        # all-scores-zero fallback: reproduce the reference's jax.random draw
        import jax

        with jax.default_device(jax.devices("cpu")[0]):
            k = jax.random.key(1)
            OP_w = np.asarray(jax.random.uniform(k, (N_OP,)), dtype=np.float64)
            CO_w = np.asarray(
                jax.random.uniform(jax.random.fold_in(k, 1), (N_CO,)),
                dtype=np.float64,
            )
        op = np.asarray(OP_zs, dtype=np.float64)[0]
        co = np.asarray(CO_zs, dtype=np.float64)[0]
        u_op, u_co = OP_w @ op, CO_w @ co
        return (
            (u_op / OP_w.sum())[None].astype(np.float32),
            (u_co / CO_w.sum())[None].astype(np.float32),
        )

    return (
        (u_op / T)[None].astype(np.float32),
        (u_co / T)[None].astype(np.float32),
    )
